# revision 1
# baseline (speedup 1.0000x reference)
"""Trainium2 Bass kernel for the unsupervised-entropy loss.

intra = mean_r H_r where H_r = entropy(softmax(-d2(x_r, m))).
Softmax is shift-invariant, so with unit-norm m rows the logits reduce to
z = 2 x m^T (the ||x||^2 and ||m||^2 terms drop).  Per row:
  S = sum_j exp(z_j),  W = sum_j z_j exp(z_j),  H = log S - W/S
(invariant to any constant logit shift, so no max-subtraction needed; z is
bounded by ~|2 x.m| <= ~13 which exp handles comfortably in fp32).

Device pipeline per core (x shard [32768,128]), per 1024-row block:
  1. SWDGE DMA-cast load f32->bf16, natural layout [128p, 8, 128]
  2. 8 PE transposes (bf16, vs identity) -> PSUM xT [128, 8, 128]
  3. evict xT PSUM->SBUF (split: ScalarE half / VectorE half)
  4. two bf16 matmuls accumulate z^T [128, 512] into one PSUM bank
     (lhsT = 2*m^T zero-padded to cols 0-63 / 64-127 -> chunk A on
     partitions 0-63, chunk B on partitions 64-127)
  5. ACT exp -> E bf16;  DVE z*E -> P bf16
  6. two reduce matmuls (block-indicator lhsT) -> S[2,512], W[2,512]
  7. evict stats (DVE copies S, ACT copies W) into [2, 32, 512] buffers
Final: SBUF->SBUF DMA rearranges stats to [128,256]; ACT Ln(+accum),
ACT exp(-lnS) for 1/S, DVE (W*rS -> accum). Output [128,2] per-partition
partial sums of log S and W/S; host reduces, adds the (tiny) inter term.
"""

import json

import numpy as np
import ml_dtypes

import concourse.bass as _bass
import concourse.tile as _tile
from concourse import mybir
from concourse.bass_utils import run_bass_kernel_spmd
from concourse.vector_clock import ScopedClock

F32 = mybir.dt.float32
BF16 = mybir.dt.bfloat16
N, D, K = 262144, 128, 64
NCORES = 8
NSHARD = N // NCORES          # 32768 rows per core
BLK = 1024                    # rows per block
NBLK = NSHARD // BLK          # 32 blocks
EPS = 1e-16
LAMB = 1.0


# ---- workarounds: this walrus build rejects >1 sync wait per instruction ----

def _split_multiwait(json_bytes: bytes) -> bytes:
    data = json.loads(json_bytes)
    counter = [0]
    for fn in data["functions"]:
        for blk in fn["blocks"]:
            new_insts = []
            for inst in blk["instructions"]:
                si = inst.get("sync_info")
                waits = (si or {}).get("on_wait") or []
                if len(waits) > 1:
                    for w in waits[:-1]:
                        counter[0] += 1
                        new_insts.append({
                            "debug": inst.get("debug"),
                            "engine": inst["engine"],
                            "ins": [],
                            "name": f"splitw_{counter[0]}_{inst['name']}",
                            "opcode": "EventSemaphore",
                            "outs": [],
                            "sync_info": {"on_update": [], "on_wait": [w]},
                        })
                    si["on_wait"] = [waits[-1]]
                new_insts.append(inst)
            blk["instructions"] = new_insts
    return json.dumps(data).encode()


class PatchedBass(_bass.Bass):
    def to_json_bytes(self) -> bytes:
        return _split_multiwait(super().to_json_bytes())


class SplitDrainTileContext(_tile.TileContext):
    def _drain_and_barrier(self, tick_clock, wait_clock):
        drain_inst = self.nc.sync.drain()
        wait_clock.add_sem_waits(
            drain_inst.ins, ScopedClock({None: tick_clock.global_clock})
        )
        si = drain_inst.ins.sync_info
        if si is not None and len(si.on_wait) > 1:
            waits = list(si.on_wait)
            si.on_wait = waits[:1]
            drain_inst.ins.sync_info = si
            for w in waits[1:]:
                d2 = self.nc.sync.drain()
                si2 = d2.ins.sync_info
                if si2 is None:
                    import copy
                    si2 = copy.copy(si)
                si2.on_wait = [w]
                si2.on_update = []
                d2.ins.sync_info = si2
        self.nc.all_engine_barrier()
        assert self.sems is not None
        popped = self.nc._tile_sem_poison_stack.pop()
        assert popped is self._sem_poison
        self.nc.clear_and_free_semaphores(list(self.sems.allocated().values()))
        self.nc.all_engine_barrier()


# ------------------------------ kernel build ------------------------------

_CACHE = {}


def _build():
    if "nc" in _CACHE:
        return _CACHE["nc"]
    nc = PatchedBass("TRN2", target_bir_lowering=False, debug=False)
    xs_ap = nc.dram_tensor("xs", [NSHARD, D], F32, kind="ExternalInput").ap()
    padA_ap = nc.dram_tensor("padA", [D, 128], BF16, kind="ExternalInput").ap()
    padB_ap = nc.dram_tensor("padB", [D, 128], BF16, kind="ExternalInput").ap()
    ind_ap = nc.dram_tensor("ind", [128, 2], BF16, kind="ExternalInput").ap()
    id_ap = nc.dram_tensor("ident", [128, 128], BF16, kind="ExternalInput").ap()
    out_ap = nc.dram_tensor("out", [128, 2], F32, kind="ExternalOutput").ap()

    Exp = mybir.ActivationFunctionType.Exp
    Ln = mybir.ActivationFunctionType.Ln
    MUL = mybir.AluOpType.mult
    ADD = mybir.AluOpType.add

    xs_v = xs_ap.rearrange("(b g p) d -> b p g d", p=128, g=BLK // 128)

    with SplitDrainTileContext(nc) as tc:
        with tc.tile_pool(name="const", bufs=1) as const, \
             tc.tile_pool(name="xin", bufs=3) as xin, \
             tc.tile_pool(name="xtp", bufs=3) as xtp, \
             tc.tile_pool(name="ep", bufs=3) as ep, \
             tc.tile_pool(name="stage", bufs=1) as stage, \
             tc.tile_pool(name="fin", bufs=1) as fin, \
             tc.tile_pool(name="psT", bufs=2, space="PSUM") as psTp, \
             tc.tile_pool(name="psZ", bufs=2, space="PSUM") as psZp, \
             tc.tile_pool(name="psS", bufs=2, space="PSUM") as psSp, \
             tc.tile_pool(name="psW", bufs=2, space="PSUM") as psWp:

            padA = const.tile([D, 128], BF16)
            nc.sync.dma_start(out=padA[:], in_=padA_ap[:])
            padB = const.tile([D, 128], BF16)
            nc.sync.dma_start(out=padB[:], in_=padB_ap[:])
            ind = const.tile([128, 2], BF16)
            nc.sync.dma_start(out=ind[:], in_=ind_ap[:])
            ident = const.tile([128, 128], BF16)
            nc.sync.dma_start(out=ident[:], in_=id_ap[:])

            stats_s = stage.tile([2, NBLK, 512], F32)
            stats_w = stage.tile([2, NBLK, 512], F32)

            G = BLK // 128  # 8 tiles per block
            for b in range(NBLK):
                xb = xin.tile([128, G, D], BF16)
                nc.gpsimd.dma_start(out=xb[:], in_=xs_v[b])

                psT = psTp.tile([128, G, 128], BF16)
                for g in range(G):
                    nc.tensor.transpose(psT[:, g, :], xb[:, g, :], ident[:])

                xT = xtp.tile([128, G, 128], BF16)
                # ScalarE takes 5 tiles, VectorE 3: balances ACT vs DVE busy
                # time (DVE also carries the z*E multiply and the S eviction).
                nc.scalar.copy(xT[:, 0:5, :], psT[:, 0:5, :])
                nc.vector.tensor_copy(xT[:, 5:G, :], psT[:, 5:G, :])
                xTf = xT[:].rearrange("p g r -> p (g r)")

                psZ = psZp.tile([128, 512], F32)
                nc.tensor.matmul(psZ[:], padA[:], xTf[:, 0:512],
                                 start=True, stop=False)
                nc.tensor.matmul(psZ[:], padB[:], xTf[:, 512:1024],
                                 start=False, stop=True)

                E = ep.tile([128, 512], BF16, tag="E")
                nc.scalar.activation(E[:], psZ[:], Exp)
                P = ep.tile([128, 512], BF16, tag="P")
                nc.vector.scalar_tensor_tensor(P[:], psZ[:], 1.0, E[:], MUL, MUL)

                psS = psSp.tile([2, 512], F32)
                nc.tensor.matmul(psS[:], ind[:], E[:], start=True, stop=True)
                psW = psWp.tile([2, 512], F32)
                nc.tensor.matmul(psW[:], ind[:], P[:], start=True, stop=True)

                nc.vector.tensor_copy(stats_s[:, b, :], psS[:])
                nc.scalar.copy(stats_w[:, b, :], psW[:])

            # final: rearrange per-row stats onto 128 partitions and reduce
            s128 = fin.tile([128, 256], F32)
            w128 = fin.tile([128, 256], F32)
            nc.sync.dma_start(out=s128[:],
                              in_=stats_s[:].rearrange("p a b -> p (a b)"))
            nc.sync.dma_start(out=w128[:],
                              in_=stats_w[:].rearrange("p a b -> p (a b)"))

            lnS = fin.tile([128, 256], F32)
            lsum = fin.tile([128, 1], F32)
            nc.scalar.activation(lnS[:], s128[:], Ln, accum_out=lsum[:])
            rS = fin.tile([128, 256], F32)
            nc.scalar.activation(rS[:], lnS[:], Exp, scale=-1.0)
            scr = fin.tile([128, 256], F32)
            wsum = fin.tile([128, 1], F32)
            nc.vector.scalar_tensor_tensor(scr[:], w128[:], 1.0, rS[:],
                                           MUL, MUL, accum_out=wsum[:])
            ob = fin.tile([128, 2], F32)
            nc.vector.tensor_copy(ob[:, 0:1], lsum[:])
            nc.vector.tensor_copy(ob[:, 1:2], wsum[:])
            nc.sync.dma_start(out=out_ap[:], in_=ob[:])

    _CACHE["nc"] = nc
    return nc


def _entropy_np(p):
    p = np.where(p <= 0, EPS, p)
    p = np.where(p >= 1, 1.0 - EPS, p)
    return -np.sum(p * np.log(p), axis=-1)


def kernel(x, m):
    nc = _build()

    mt2 = (2.0 * np.float64(m).T).astype(np.float32)       # [128, 64]
    padA = np.zeros((D, 128), dtype=ml_dtypes.bfloat16)
    padB = np.zeros((D, 128), dtype=ml_dtypes.bfloat16)
    padA[:, 0:K] = mt2.astype(ml_dtypes.bfloat16)
    padB[:, K:128] = mt2.astype(ml_dtypes.bfloat16)
    ind = np.zeros((128, 2), dtype=ml_dtypes.bfloat16)
    ind[0:K, 0] = 1
    ind[K:128, 1] = 1
    ident = np.eye(128, dtype=ml_dtypes.bfloat16)

    in_maps = []
    for c in range(NCORES):
        in_maps.append({
            "xs": np.ascontiguousarray(x[c * NSHARD:(c + 1) * NSHARD]),
            "padA": padA, "padB": padB, "ind": ind, "ident": ident,
        })
    _CACHE["last_in_maps"] = in_maps
    res = run_bass_kernel_spmd(nc, in_maps, core_ids=list(range(NCORES)))

    tot_ls = 0.0
    tot_ws = 0.0
    for c in range(NCORES):
        o = np.float64(res.results[c]["out"])
        tot_ls += o[:, 0].sum()
        tot_ws += o[:, 1].sum()
    intra = (tot_ls - tot_ws) / N

    # inter term on host (tiny), replicating the reference exactly
    m64 = np.float64(m)
    mu = m64.mean(axis=0)
    d2 = ((mu[None, :] - m64) ** 2).sum(axis=1)
    zl = -d2
    zl -= zl.max()
    e = np.exp(zl)
    p = e / e.sum()
    inter = _entropy_np(p)

    total = intra - LAMB * inter
    return (np.float32(total), np.float32(intra), np.float32(inter))



# revision 3
# speedup vs baseline: 1.4225x; 1.4225x over previous
"""Trainium2 Bass kernel for the unsupervised-entropy loss.

intra = mean_r H_r where H_r = entropy(softmax(-d2(x_r, m))).
Softmax is shift-invariant, so with unit-norm m rows the logits reduce to
z = 2 x m^T (the ||x||^2 and ||m||^2 terms drop).  Per row:
  S = sum_j exp(z_j),  W = sum_j z_j exp(z_j),  H = log S - W/S

Row-to-partition assignment is contiguous (partition p owns rows
[p*256, (p+1)*256) of the shard) so every DMA chunk is one contiguous
8 KiB read per partition (128 descriptors per chunk, minimal SWDGE cost).
Row order is irrelevant: only sums over all rows are needed.

Software-pipelined device loop, per iteration i (block b = 1024 rows):
  PE : z-matmuls(b=i-1)  [col-tiled pair: chunk A -> psum partitions 0:64,
       chunk B -> 64:128, one accumulation group, concurrent col-groups]
       8 PE transposes(b=i)   x_nat [128p,128d] -> psum xT [128d,128p]
       2 reduce matmuls(b=i-2): indicator lhsT [128,16] accumulating
       S and W into one psum bank [16,512] shared by 4 blocks
  ACT: exp(psZ(i-1)) -> E bf16; evict xT tiles 0:3 psum->sbuf
  DVE: P = z*E (b=i-2); evict xT tiles 3:8; nothing else
  ACT: every 4th block: evict the [16,512] S/W bank -> bf16 staging
  GpSimd: one SWDGE cast-load (f32->bf16) of a 1 MiB chunk every 2 blocks

Final: two SBUF->SBUF DMAs fan the staged [8,8,512] S and W values onto
[128,256]; ACT Ln(+accum lsum), ACT exp(-lnS)=1/S, DVE W*rS(+accum wsum).
Output [128,2] per-partition partial sums; host reduces and adds the
(tiny) inter term.
"""

import json

import numpy as np
import ml_dtypes

import concourse.bass as _bass
import concourse.tile as _tile
from concourse import mybir
from concourse.bass_utils import run_bass_kernel_spmd
from concourse.vector_clock import ScopedClock

F32 = mybir.dt.float32
BF16 = mybir.dt.bfloat16
N, D, K = 262144, 128, 64
NCORES = 8
NSHARD = N // NCORES          # 32768 rows per core
BLK = 1024                    # rows per block
NBLK = NSHARD // BLK          # 32 blocks
NCHUNK = 16                   # DMA chunks (2 blocks each)
RPP = NSHARD // 128           # rows per partition (256)
EPS = 1e-16
LAMB = 1.0


# ---- workarounds: this walrus build rejects >1 sync wait per instruction ----

def _split_multiwait(json_bytes: bytes) -> bytes:
    data = json.loads(json_bytes)
    counter = [0]
    for fn in data["functions"]:
        for blk in fn["blocks"]:
            new_insts = []
            for inst in blk["instructions"]:
                si = inst.get("sync_info")
                waits = (si or {}).get("on_wait") or []
                if len(waits) > 1:
                    for w in waits[:-1]:
                        counter[0] += 1
                        new_insts.append({
                            "debug": inst.get("debug"),
                            "engine": inst["engine"],
                            "ins": [],
                            "name": f"splitw_{counter[0]}_{inst['name']}",
                            "opcode": "EventSemaphore",
                            "outs": [],
                            "sync_info": {"on_update": [], "on_wait": [w]},
                        })
                    si["on_wait"] = [waits[-1]]
                new_insts.append(inst)
            blk["instructions"] = new_insts
    return json.dumps(data).encode()


class PatchedBass(_bass.Bass):
    def to_json_bytes(self) -> bytes:
        return _split_multiwait(super().to_json_bytes())


class SplitDrainTileContext(_tile.TileContext):
    def _drain_and_barrier(self, tick_clock, wait_clock):
        drain_inst = self.nc.sync.drain()
        wait_clock.add_sem_waits(
            drain_inst.ins, ScopedClock({None: tick_clock.global_clock})
        )
        si = drain_inst.ins.sync_info
        if si is not None and len(si.on_wait) > 1:
            waits = list(si.on_wait)
            si.on_wait = waits[:1]
            drain_inst.ins.sync_info = si
            for w in waits[1:]:
                d2 = self.nc.sync.drain()
                si2 = d2.ins.sync_info
                if si2 is None:
                    import copy
                    si2 = copy.copy(si)
                si2.on_wait = [w]
                si2.on_update = []
                d2.ins.sync_info = si2
        self.nc.all_engine_barrier()
        assert self.sems is not None
        popped = self.nc._tile_sem_poison_stack.pop()
        assert popped is self._sem_poison
        self.nc.clear_and_free_semaphores(list(self.sems.allocated().values()))
        self.nc.all_engine_barrier()


# ------------------------------ kernel build ------------------------------

_CACHE = {}


def _build():
    if "nc" in _CACHE:
        return _CACHE["nc"]
    nc = PatchedBass("TRN2", target_bir_lowering=False, debug=False)
    xs_ap = nc.dram_tensor("xs", [NSHARD, D], F32, kind="ExternalInput").ap()
    m2t_ap = nc.dram_tensor("m2t", [D, K], BF16, kind="ExternalInput").ap()
    ind_ap = nc.dram_tensor("ind", [128, 8, 16], BF16, kind="ExternalInput").ap()
    id_ap = nc.dram_tensor("ident", [128, 128], BF16, kind="ExternalInput").ap()
    out_ap = nc.dram_tensor("out", [128, 2], F32, kind="ExternalOutput").ap()

    Exp = mybir.ActivationFunctionType.Exp
    Ln = mybir.ActivationFunctionType.Ln
    MUL = mybir.AluOpType.mult

    # row = p*(NCHUNK*CR) + c*CR + r  with CR rows per chunk per partition
    CR = RPP // NCHUNK            # 16 rows per partition per chunk
    xsv = xs_ap.rearrange("(p c r) d -> c p r d", p=128, c=NCHUNK)

    with SplitDrainTileContext(nc) as tc:
        with tc.tile_pool(name="const", bufs=1) as const, \
             tc.tile_pool(name="xin", bufs=3) as xin, \
             tc.tile_pool(name="xtp", bufs=3) as xtp, \
             tc.tile_pool(name="ep", bufs=3) as ep, \
             tc.tile_pool(name="pp", bufs=2) as pp, \
             tc.tile_pool(name="stage", bufs=1) as stage, \
             tc.tile_pool(name="fin", bufs=1) as fin, \
             tc.tile_pool(name="psT", bufs=3, space="PSUM") as psTp, \
             tc.tile_pool(name="psZ", bufs=3, space="PSUM") as psZp, \
             tc.tile_pool(name="psSW", bufs=2, space="PSUM") as psSWp:

            m2t = const.tile([D, K], BF16)
            nc.sync.dma_start(out=m2t[:], in_=m2t_ap[:])
            ind = const.tile([128, 8, 16], BF16)
            nc.sync.dma_start(out=ind[:], in_=ind_ap[:])
            ident = const.tile([128, 128], BF16)
            nc.sync.dma_start(out=ident[:], in_=id_ap[:])

            stats_sw = stage.tile([16, 8, 512], BF16)

            xc_t = {}     # chunk -> xin tile
            xT_t = {}     # block -> xtp tile (sbuf xT)
            psZ_t = {}    # block -> psum z tile
            E_t = {}      # block -> E tile
            psSW_t = {}   # group -> psum stats tile

            for i in range(NBLK + 2):
                # ---- PE: z-matmuls for block i-1 ----
                if 1 <= i <= NBLK:
                    b = i - 1
                    xTf = xT_t[b][:].rearrange("d r p -> d (r p)")
                    psZ = psZp.tile([128, 512], F32)
                    psZ_t[b] = psZ
                    nc.tensor.matmul(psZ[0:64, :], m2t[:], xTf[:, 0:512],
                                     start=True, stop=False,
                                     tile_position=(0, 0))
                    nc.tensor.matmul(psZ[64:128, :], m2t[:], xTf[:, 512:1024],
                                     start=False, stop=True,
                                     tile_position=(0, 64))

                # ---- ACT: exp for block i-1 ----
                if 1 <= i <= NBLK:
                    b = i - 1
                    E = ep.tile([128, 512], BF16)
                    E_t[b] = E
                    nc.scalar.activation(E[:], psZ_t[b][:], Exp)

                # ---- DVE: P = z*E for block i-2 ----
                if 2 <= i <= NBLK + 1:
                    b2 = i - 2
                    P = pp.tile([128, 512], BF16)
                    nc.vector.scalar_tensor_tensor(P[:], psZ_t[b2][:], 1.0,
                                                   E_t[b2][:], MUL, MUL)

                # ---- GpSimd: chunk load (2 blocks per chunk) ----
                if i < NBLK and i % 2 == 0:
                    c = i // 2
                    xc = xin.tile([128, CR, D], BF16)
                    xc_t[c] = xc
                    nc.gpsimd.dma_start(out=xc[:], in_=xsv[c])

                # ---- PE: 8 transposes for block i ----
                if i < NBLK:
                    c, h = divmod(i, 2)
                    xc = xc_t[c]
                    psT = psTp.tile([128, 8, 128], BF16)
                    for r in range(8):
                        nc.tensor.transpose(psT[:, r, :],
                                            xc[:, 8 * h + r, :], ident[:])
                    xT = xtp.tile([128, 8, 128], BF16)
                    xT_t[i] = xT
                    # ACT evicts 3 tiles, DVE 5 (balances engine busy)
                    nc.scalar.copy(xT[:, 0:3, :], psT[:, 0:3, :])
                    nc.vector.tensor_copy(xT[:, 3:8, :], psT[:, 3:8, :])

                # ---- PE: reduce matmuls for block i-2 ----
                if 2 <= i <= NBLK + 1:
                    b2 = i - 2
                    g, j = divmod(b2, 4)
                    if j == 0:
                        psSW_t[g] = psSWp.tile([16, 512], F32, name="psSW")
                    psSW = psSW_t[g]
                    nc.tensor.matmul(psSW[:], ind[:, j, :], E_t[b2][:],
                                     start=(j == 0), stop=False,
                                     skip_group_check=True)
                    nc.tensor.matmul(psSW[:], ind[:, 4 + j, :], P[:],
                                     start=False, stop=(j == 3),
                                     skip_group_check=True)
                    if j == 3:
                        nc.scalar.copy(stats_sw[:, g, :], psSW[:])

                # free refs we no longer need (python-side bookkeeping only)
                if i >= 3:
                    b3 = i - 3
                    psZ_t.pop(b3, None)
                    E_t.pop(b3, None)
                    xT_t.pop(b3, None)

            # ---- final: fan staged stats to 128 partitions and reduce ----
            s128 = fin.tile([128, 256], BF16)
            w128 = fin.tile([128, 256], BF16)
            nc.sync.dma_start(
                out=s128[:],
                in_=stats_sw[0:8, :, :].rearrange("p g n -> p (g n)"))
            nc.sync.dma_start(
                out=w128[:],
                in_=stats_sw[8:16, :, :].rearrange("p g n -> p (g n)"))

            lnS = fin.tile([128, 256], F32)
            lsum = fin.tile([128, 1], F32)
            nc.scalar.activation(lnS[:], s128[:], Ln, accum_out=lsum[:])
            rS = fin.tile([128, 256], F32)
            nc.scalar.activation(rS[:], lnS[:], Exp, scale=-1.0)
            scr = fin.tile([128, 256], F32)
            wsum = fin.tile([128, 1], F32)
            nc.vector.scalar_tensor_tensor(scr[:], w128[:], 1.0, rS[:],
                                           MUL, MUL, accum_out=wsum[:])
            ob = fin.tile([128, 2], F32)
            nc.vector.tensor_copy(ob[:, 0:1], lsum[:])
            nc.vector.tensor_copy(ob[:, 1:2], wsum[:])
            nc.sync.dma_start(out=out_ap[:], in_=ob[:])

    _CACHE["nc"] = nc
    return nc


def _entropy_np(p):
    p = np.where(p <= 0, EPS, p)
    p = np.where(p >= 1, 1.0 - EPS, p)
    return -np.sum(p * np.log(p), axis=-1)


def kernel(x, m):
    nc = _build()

    m2t = (2.0 * np.float64(m).T).astype(ml_dtypes.bfloat16)   # [128, 64]
    ident = np.eye(128, dtype=ml_dtypes.bfloat16)
    ind = np.zeros((128, 8, 16), dtype=ml_dtypes.bfloat16)
    for j in range(4):
        ind[0:64, j, 2 * j] = 1          # S, chunk A (psum rows 0:8)
        ind[64:128, j, 2 * j + 1] = 1    # S, chunk B
        ind[0:64, 4 + j, 8 + 2 * j] = 1      # W, chunk A (psum rows 8:16)
        ind[64:128, 4 + j, 8 + 2 * j + 1] = 1

    in_maps = []
    for c in range(NCORES):
        in_maps.append({
            "xs": np.ascontiguousarray(x[c * NSHARD:(c + 1) * NSHARD]),
            "m2t": m2t, "ind": ind, "ident": ident,
        })
    _CACHE["last_in_maps"] = in_maps
    res = run_bass_kernel_spmd(nc, in_maps, core_ids=list(range(NCORES)))

    tot_ls = 0.0
    tot_ws = 0.0
    for c in range(NCORES):
        o = np.float64(res.results[c]["out"])
        tot_ls += o[:, 0].sum()
        tot_ws += o[:, 1].sum()
    intra = (tot_ls - tot_ws) / N

    # inter term on host (tiny), replicating the reference exactly
    m64 = np.float64(m)
    mu = m64.mean(axis=0)
    d2 = ((mu[None, :] - m64) ** 2).sum(axis=1)
    zl = -d2
    zl -= zl.max()
    e = np.exp(zl)
    p = e / e.sum()
    inter = _entropy_np(p)

    total = intra - LAMB * inter
    return (np.float32(total), np.float32(intra), np.float32(inter))


# revision 6
# speedup vs baseline: 1.5635x; 1.0991x over previous
"""Trainium2 Bass kernel for the unsupervised-entropy loss.

intra = mean_r H_r where H_r = entropy(softmax(-d2(x_r, m))).
Softmax is shift-invariant, so with unit-norm m rows the logits reduce to
z = 2 x m^T (the ||x||^2 and ||m||^2 terms drop).  Per row:
  S = sum_j exp(z_j),  W = sum_j z_j exp(z_j),  H = log S - W/S

Row-to-partition assignment is contiguous (partition p owns rows
[p*256, (p+1)*256) of the shard) so every DMA chunk is one contiguous
8 KiB read per partition (128 descriptors per chunk, minimal SWDGE cost).
Row order is irrelevant: only sums over all rows are needed.

Software-pipelined device loop, per iteration i (block b = 1024 rows):
  PE : z-matmuls(b=i-1)  [col-tiled pair: chunk A -> psum partitions 0:64,
       chunk B -> 64:128, one accumulation group, concurrent col-groups]
       8 PE transposes(b=i)   x_nat [128p,128d] -> psum xT [128d,128p]
       2 reduce matmuls(b=i-2): indicator lhsT [128,16] accumulating
       S and W into one psum bank [16,512] shared by 4 blocks
  ACT: exp(psZ(i-1)) -> E bf16; evict xT tiles 0:3 psum->sbuf
  DVE: P = z*E (b=i-2); evict xT tiles 3:8; nothing else
  ACT: every 4th block: evict the [16,512] S/W bank -> bf16 staging
  GpSimd: one SWDGE cast-load (f32->bf16) of a 1 MiB chunk every 2 blocks

Final: two SBUF->SBUF DMAs fan the staged [8,8,512] S and W values onto
[128,256]; ACT Ln(+accum lsum), ACT exp(-lnS)=1/S, DVE W*rS(+accum wsum).
Output [128,2] per-partition partial sums; host reduces and adds the
(tiny) inter term.
"""

import json

import numpy as np
import ml_dtypes

import concourse.bass as _bass
import concourse.tile as _tile
from concourse import mybir
from concourse.bass_utils import run_bass_kernel_spmd
from concourse.vector_clock import ScopedClock

F32 = mybir.dt.float32
BF16 = mybir.dt.bfloat16
N, D, K = 262144, 128, 64
NCORES = 8
NSHARD = N // NCORES          # 32768 rows per core
BLK = 1024                    # rows per block
NBLK = NSHARD // BLK          # 32 blocks
NCHUNK = 16                   # DMA chunks (2 blocks each)
RPP = NSHARD // 128           # rows per partition (256)
EPS = 1e-16
LAMB = 1.0


# ---- workarounds: this walrus build rejects >1 sync wait per instruction ----

def _split_multiwait(json_bytes: bytes) -> bytes:
    data = json.loads(json_bytes)
    counter = [0]
    for fn in data["functions"]:
        for blk in fn["blocks"]:
            new_insts = []
            for inst in blk["instructions"]:
                si = inst.get("sync_info")
                waits = (si or {}).get("on_wait") or []
                if len(waits) > 1:
                    for w in waits[:-1]:
                        counter[0] += 1
                        new_insts.append({
                            "debug": inst.get("debug"),
                            "engine": inst["engine"],
                            "ins": [],
                            "name": f"splitw_{counter[0]}_{inst['name']}",
                            "opcode": "EventSemaphore",
                            "outs": [],
                            "sync_info": {"on_update": [], "on_wait": [w]},
                        })
                    si["on_wait"] = [waits[-1]]
                new_insts.append(inst)
            blk["instructions"] = new_insts
    return json.dumps(data).encode()


class PatchedBass(_bass.Bass):
    def to_json_bytes(self) -> bytes:
        return _split_multiwait(super().to_json_bytes())


class SplitDrainTileContext(_tile.TileContext):
    def _drain_and_barrier(self, tick_clock, wait_clock):
        drain_inst = self.nc.sync.drain()
        wait_clock.add_sem_waits(
            drain_inst.ins, ScopedClock({None: tick_clock.global_clock})
        )
        si = drain_inst.ins.sync_info
        if si is not None and len(si.on_wait) > 1:
            waits = list(si.on_wait)
            si.on_wait = waits[:1]
            drain_inst.ins.sync_info = si
            for w in waits[1:]:
                d2 = self.nc.sync.drain()
                si2 = d2.ins.sync_info
                if si2 is None:
                    import copy
                    si2 = copy.copy(si)
                si2.on_wait = [w]
                si2.on_update = []
                d2.ins.sync_info = si2
        self.nc.all_engine_barrier()
        assert self.sems is not None
        popped = self.nc._tile_sem_poison_stack.pop()
        assert popped is self._sem_poison
        self.nc.clear_and_free_semaphores(list(self.sems.allocated().values()))
        self.nc.all_engine_barrier()


# ------------------------------ kernel build ------------------------------

_CACHE = {}


def _build():
    if "nc" in _CACHE:
        return _CACHE["nc"]
    nc = PatchedBass("TRN2", target_bir_lowering=False, debug=False)
    xs_ap = nc.dram_tensor("xs", [NSHARD, D], F32, kind="ExternalInput").ap()
    m2t_ap = nc.dram_tensor("m2t", [D, K], BF16, kind="ExternalInput").ap()
    ind_ap = nc.dram_tensor("ind", [128, 8, 16], BF16, kind="ExternalInput").ap()
    id_ap = nc.dram_tensor("ident", [128, 128], BF16, kind="ExternalInput").ap()
    out_ap = nc.dram_tensor("out", [128, 2], F32, kind="ExternalOutput").ap()

    Exp = mybir.ActivationFunctionType.Exp
    Ln = mybir.ActivationFunctionType.Ln
    MUL = mybir.AluOpType.mult

    # row = p*(NCHUNK*CR) + c*CR + r  with CR rows per chunk per partition
    CR = RPP // NCHUNK            # 16 rows per partition per chunk
    xsv = xs_ap.rearrange("(p c r) d -> c p r d", p=128, c=NCHUNK)

    with SplitDrainTileContext(nc) as tc:
        with tc.tile_pool(name="const", bufs=1) as const, \
             tc.tile_pool(name="xin", bufs=3) as xin, \
             tc.tile_pool(name="xtp", bufs=3) as xtp, \
             tc.tile_pool(name="ep", bufs=3) as ep, \
             tc.tile_pool(name="pp", bufs=2) as pp, \
             tc.tile_pool(name="stage", bufs=1) as stage, \
             tc.tile_pool(name="fin", bufs=1) as fin, \
             tc.tile_pool(name="psT", bufs=3, space="PSUM") as psTp, \
             tc.tile_pool(name="psZ", bufs=3, space="PSUM") as psZp, \
             tc.tile_pool(name="psSW", bufs=2, space="PSUM") as psSWp:

            m2t = const.tile([D, K], BF16)
            nc.sync.dma_start(out=m2t[:], in_=m2t_ap[:])
            ind = const.tile([128, 8, 16], BF16)
            nc.sync.dma_start(out=ind[:], in_=ind_ap[:])
            ident = const.tile([128, 128], BF16)
            nc.sync.dma_start(out=ident[:], in_=id_ap[:])

            stats_sw = stage.tile([16, 8, 512], BF16)

            xc_t = {}     # chunk -> xin tile
            xT_t = {}     # block -> xtp tile (sbuf xT)
            psZ_t = {}    # block -> psum z tile
            E_t = {}      # block -> E tile
            psSW_t = {}   # group -> psum stats tile

            for i in range(NBLK + 2):
                # ---- PE: z-matmuls for block i-1 ----
                if 1 <= i <= NBLK:
                    b = i - 1
                    xTf = xT_t[b][:].rearrange("d r p -> d (r p)")
                    psZ = psZp.tile([128, 512], F32)
                    psZ_t[b] = psZ
                    nc.tensor.matmul(psZ[0:64, :], m2t[:], xTf[:, 0:512],
                                     start=True, stop=True,
                                     tile_position=(0, 0))
                    nc.tensor.matmul(psZ[64:128, :], m2t[:], xTf[:, 512:1024],
                                     start=True, stop=True,
                                     tile_position=(0, 64))

                # ---- ACT: exp for block i-1 ----
                if 1 <= i <= NBLK:
                    b = i - 1
                    E = ep.tile([128, 512], BF16)
                    E_t[b] = E
                    nc.scalar.activation(E[:], psZ_t[b][:], Exp)

                # ---- DVE: P = z*E for block i-2 ----
                if 2 <= i <= NBLK + 1:
                    b2 = i - 2
                    P = pp.tile([128, 512], BF16)
                    nc.vector.scalar_tensor_tensor(P[:], psZ_t[b2][:], 1.0,
                                                   E_t[b2][:], MUL, MUL)

                # ---- GpSimd: chunk load (2 blocks per chunk) ----
                if i < NBLK and i % 2 == 0:
                    c = i // 2
                    xc = xin.tile([128, CR, D], BF16)
                    xc_t[c] = xc
                    nc.gpsimd.dma_start(out=xc[:], in_=xsv[c])

                # ---- PE: 8 transposes for block i ----
                if i < NBLK:
                    c, h = divmod(i, 2)
                    xc = xc_t[c]
                    psT = psTp.tile([128, 8, 128], BF16)
                    for r in range(8):
                        nc.tensor.transpose(psT[:, r, :],
                                            xc[:, 8 * h + r, :], ident[:])
                    xT = xtp.tile([128, 8, 128], BF16)
                    xT_t[i] = xT
                    # ACT evicts 3 tiles, DVE 5 (balances engine busy)
                    nc.scalar.copy(xT[:, 0:3, :], psT[:, 0:3, :])
                    nc.vector.tensor_copy(xT[:, 3:8, :], psT[:, 3:8, :])

                # ---- PE: reduce matmuls for block i-2 ----
                if 2 <= i <= NBLK + 1:
                    b2 = i - 2
                    g, j = divmod(b2, 4)
                    if j == 0:
                        psSW_t[g] = psSWp.tile([16, 512], F32, name="psSW")
                    psSW = psSW_t[g]
                    # j==0 S-matmul writes the full [16,512] region (zeros
                    # outside its rows), setting has_written everywhere, so
                    # the remaining 7 matmuls accumulate onto clean zeros.
                    nc.tensor.matmul(psSW[:], ind[:, j, :], E_t[b2][:],
                                     start=(j == 0), stop=False,
                                     skip_group_check=True)
                    nc.tensor.matmul(psSW[:], ind[:, 4 + j, :], P[:],
                                     start=False, stop=(j == 3),
                                     skip_group_check=True)
                    if j == 3:
                        nc.scalar.copy(stats_sw[:, g, :], psSW[:])

                # free refs we no longer need (python-side bookkeeping only)
                if i >= 3:
                    b3 = i - 3
                    psZ_t.pop(b3, None)
                    E_t.pop(b3, None)
                    xT_t.pop(b3, None)

            # ---- final: fan staged stats to 128 partitions and reduce ----
            s128 = fin.tile([128, 256], BF16)
            w128 = fin.tile([128, 256], BF16)
            nc.sync.dma_start(
                out=s128[:],
                in_=stats_sw[0:8, :, :].rearrange("p g n -> p (g n)"))
            nc.sync.dma_start(
                out=w128[:],
                in_=stats_sw[8:16, :, :].rearrange("p g n -> p (g n)"))

            lnS = fin.tile([128, 256], F32)
            lsum = fin.tile([128, 1], F32)
            nc.scalar.activation(lnS[:], s128[:], Ln, accum_out=lsum[:])
            rS = fin.tile([128, 256], F32)
            nc.scalar.activation(rS[:], lnS[:], Exp, scale=-1.0)
            scr = fin.tile([128, 256], F32)
            wsum = fin.tile([128, 1], F32)
            nc.vector.scalar_tensor_tensor(scr[:], w128[:], 1.0, rS[:],
                                           MUL, MUL, accum_out=wsum[:])
            ob = fin.tile([128, 2], F32)
            nc.vector.tensor_copy(ob[:, 0:1], lsum[:])
            nc.vector.tensor_copy(ob[:, 1:2], wsum[:])
            nc.sync.dma_start(out=out_ap[:], in_=ob[:])

    _CACHE["nc"] = nc
    return nc


def _entropy_np(p):
    p = np.where(p <= 0, EPS, p)
    p = np.where(p >= 1, 1.0 - EPS, p)
    return -np.sum(p * np.log(p), axis=-1)


def kernel(x, m):
    nc = _build()

    m2t = (2.0 * np.float64(m).T).astype(ml_dtypes.bfloat16)   # [128, 64]
    ident = np.eye(128, dtype=ml_dtypes.bfloat16)
    ind = np.zeros((128, 8, 16), dtype=ml_dtypes.bfloat16)
    for j in range(4):
        ind[0:64, j, 2 * j] = 1          # S, chunk A (psum rows 0:8)
        ind[64:128, j, 2 * j + 1] = 1    # S, chunk B
        ind[0:64, 4 + j, 8 + 2 * j] = 1      # W, chunk A (psum rows 8:16)
        ind[64:128, 4 + j, 8 + 2 * j + 1] = 1

    in_maps = []
    for c in range(NCORES):
        in_maps.append({
            "xs": np.ascontiguousarray(x[c * NSHARD:(c + 1) * NSHARD]),
            "m2t": m2t, "ind": ind, "ident": ident,
        })
    _CACHE["last_in_maps"] = in_maps
    res = run_bass_kernel_spmd(nc, in_maps, core_ids=list(range(NCORES)))

    tot_ls = 0.0
    tot_ws = 0.0
    for c in range(NCORES):
        o = np.float64(res.results[c]["out"])
        tot_ls += o[:, 0].sum()
        tot_ws += o[:, 1].sum()
    intra = (tot_ls - tot_ws) / N

    # inter term on host (tiny), replicating the reference exactly
    m64 = np.float64(m)
    mu = m64.mean(axis=0)
    d2 = ((mu[None, :] - m64) ** 2).sum(axis=1)
    zl = -d2
    zl -= zl.max()
    e = np.exp(zl)
    p = e / e.sum()
    inter = _entropy_np(p)

    total = intra - LAMB * inter
    return (np.float32(total), np.float32(intra), np.float32(inter))


# revision 10
# speedup vs baseline: 1.5750x; 1.0074x over previous
"""Trainium2 Bass kernel for the unsupervised-entropy loss.

intra = mean_r H_r where H_r = entropy(softmax(-d2(x_r, m))).
Softmax is shift-invariant, so with unit-norm m rows the logits reduce to
z = 2 x m^T (the ||x||^2 and ||m||^2 terms drop).  Per row:
  S = sum_j exp(z_j),  W = sum_j z_j exp(z_j),  H = log S - W/S

Row-to-partition assignment is contiguous (partition p owns rows
[p*256, (p+1)*256) of the shard) so every DMA chunk is one contiguous
8 KiB read per partition (128 descriptors per chunk, minimal SWDGE cost).
Row order is irrelevant: only sums over all rows are needed.

Software-pipelined device loop, per iteration i (block b = 1024 rows):
  PE : z-matmuls(b=i-1)  [col-tiled pair: chunk A -> psum partitions 0:64,
       chunk B -> 64:128, one accumulation group, concurrent col-groups]
       8 PE transposes(b=i)   x_nat [128p,128d] -> psum xT [128d,128p]
       2 reduce matmuls(b=i-2): indicator lhsT [128,16] accumulating
       S and W into one psum bank [16,512] shared by 4 blocks
  ACT: exp(psZ(i-1)) -> E bf16; evict xT tiles 0:3 psum->sbuf
  DVE: P = z*E (b=i-2); evict xT tiles 3:8; nothing else
  ACT: every 4th block: evict the [16,512] S/W bank -> bf16 staging
  GpSimd: one SWDGE cast-load (f32->bf16) of a 1 MiB chunk every 2 blocks

Final: two SBUF->SBUF DMAs fan the staged [8,8,512] S and W values onto
[128,256]; ACT Ln(+accum lsum), ACT exp(-lnS)=1/S, DVE W*rS(+accum wsum).
Output [128,2] per-partition partial sums; host reduces and adds the
(tiny) inter term.
"""

import json

import numpy as np
import ml_dtypes

import concourse.bass as _bass
import concourse.tile as _tile
from concourse import mybir
from concourse.bass_utils import run_bass_kernel_spmd
from concourse.vector_clock import ScopedClock

F32 = mybir.dt.float32
BF16 = mybir.dt.bfloat16
N, D, K = 262144, 128, 64
NCORES = 8
NSHARD = N // NCORES          # 32768 rows per core
BLK = 1024                    # rows per block
NBLK = NSHARD // BLK          # 32 blocks
NCHUNK = 16                   # DMA chunks (2 blocks each)
RPP = NSHARD // 128           # rows per partition (256)
EPS = 1e-16
LAMB = 1.0


# ---- workarounds: this walrus build rejects >1 sync wait per instruction ----

def _split_multiwait(json_bytes: bytes) -> bytes:
    data = json.loads(json_bytes)
    counter = [0]
    for fn in data["functions"]:
        for blk in fn["blocks"]:
            new_insts = []
            for inst in blk["instructions"]:
                si = inst.get("sync_info")
                waits = (si or {}).get("on_wait") or []
                if len(waits) > 1:
                    for w in waits[:-1]:
                        counter[0] += 1
                        new_insts.append({
                            "debug": inst.get("debug"),
                            "engine": inst["engine"],
                            "ins": [],
                            "name": f"splitw_{counter[0]}_{inst['name']}",
                            "opcode": "EventSemaphore",
                            "outs": [],
                            "sync_info": {"on_update": [], "on_wait": [w]},
                        })
                    si["on_wait"] = [waits[-1]]
                new_insts.append(inst)
            blk["instructions"] = new_insts
    return json.dumps(data).encode()


class PatchedBass(_bass.Bass):
    def to_json_bytes(self) -> bytes:
        return _split_multiwait(super().to_json_bytes())


class SplitDrainTileContext(_tile.TileContext):
    def _drain_and_barrier(self, tick_clock, wait_clock):
        drain_inst = self.nc.sync.drain()
        wait_clock.add_sem_waits(
            drain_inst.ins, ScopedClock({None: tick_clock.global_clock})
        )
        si = drain_inst.ins.sync_info
        if si is not None and len(si.on_wait) > 1:
            waits = list(si.on_wait)
            si.on_wait = waits[:1]
            drain_inst.ins.sync_info = si
            for w in waits[1:]:
                d2 = self.nc.sync.drain()
                si2 = d2.ins.sync_info
                if si2 is None:
                    import copy
                    si2 = copy.copy(si)
                si2.on_wait = [w]
                si2.on_update = []
                d2.ins.sync_info = si2
        self.nc.all_engine_barrier()
        assert self.sems is not None
        popped = self.nc._tile_sem_poison_stack.pop()
        assert popped is self._sem_poison
        self.nc.clear_and_free_semaphores(list(self.sems.allocated().values()))
        self.nc.all_engine_barrier()


# ------------------------------ kernel build ------------------------------

_CACHE = {}


def _build():
    if "nc" in _CACHE:
        return _CACHE["nc"]
    nc = PatchedBass("TRN2", target_bir_lowering=False, debug=False)
    xs_ap = nc.dram_tensor("xs", [NSHARD, D], F32, kind="ExternalInput").ap()
    m2t_ap = nc.dram_tensor("m2t", [D, K], BF16, kind="ExternalInput").ap()
    ind_ap = nc.dram_tensor("ind", [128, 8, 8], BF16, kind="ExternalInput").ap()
    id_ap = nc.dram_tensor("ident", [128, 128], BF16, kind="ExternalInput").ap()
    out_ap = nc.dram_tensor("out", [128, 2], F32, kind="ExternalOutput").ap()

    Exp = mybir.ActivationFunctionType.Exp
    Ln = mybir.ActivationFunctionType.Ln
    MUL = mybir.AluOpType.mult

    # row = p*(NCHUNK*CR) + c*CR + r  with CR rows per chunk per partition
    CR = RPP // NCHUNK            # 16 rows per partition per chunk
    xsv = xs_ap.rearrange("(p c r) d -> c p r d", p=128, c=NCHUNK)

    with SplitDrainTileContext(nc) as tc:
        with tc.tile_pool(name="const", bufs=1) as const, \
             tc.tile_pool(name="xin", bufs=3) as xin, \
             tc.tile_pool(name="xtp", bufs=4) as xtp, \
             tc.tile_pool(name="ep", bufs=4) as ep, \
             tc.tile_pool(name="pp", bufs=3) as pp, \
             tc.tile_pool(name="stage", bufs=1) as stage, \
             tc.tile_pool(name="fin", bufs=1) as fin, \
             tc.tile_pool(name="psT", bufs=2, space="PSUM") as psTp, \
             tc.tile_pool(name="psZ", bufs=3, space="PSUM") as psZp, \
             tc.tile_pool(name="psSW", bufs=2, space="PSUM") as psSWp:

            m2t = const.tile([D, K], BF16)
            nc.sync.dma_start(out=m2t[:], in_=m2t_ap[:])
            ind = const.tile([128, 8, 8], BF16)
            nc.sync.dma_start(out=ind[:], in_=ind_ap[:])
            ident = const.tile([128, 128], BF16)
            nc.sync.dma_start(out=ident[:], in_=id_ap[:])

            # staged S/W per 4-block group (rows 0:8 = S, 32:40 = W,
            # rows 8:32 are dead padding so one evict instruction covers
            # both at the same per-partition free-dim cost)
            stats_sw = stage.tile([40, 8, 512], BF16)
            s128 = fin.tile([128, 8, 32], BF16)
            w128 = fin.tile([128, 8, 32], BF16)

            xc_t = {}     # chunk -> xin tile
            xT_t = {}     # block -> xtp tile (sbuf xT)
            psZ_t = {}    # block -> psum z tile
            E_t = {}      # block -> E tile
            P_t = {}      # block -> P tile
            psSW_t = {}   # group -> psum stats tile

            for i in range(NBLK + 4):
                # ---- PE: z-matmuls for block i-2 (xT evicted 2 iters ago) ----
                if 2 <= i <= NBLK + 1:
                    b = i - 2
                    xTf = xT_t[b][:].rearrange("d r p -> d (r p)")
                    psZ = psZp.tile([128, 512], F32)
                    psZ_t[b] = psZ
                    nc.tensor.matmul(psZ[0:64, :], m2t[:], xTf[:, 0:512],
                                     start=True, stop=True,
                                     tile_position=(0, 0))
                    nc.tensor.matmul(psZ[64:128, :], m2t[:], xTf[:, 512:1024],
                                     start=True, stop=True,
                                     tile_position=(0, 64))

                # ---- ACT: exp for block i-2 ----
                if 2 <= i <= NBLK + 1:
                    b = i - 2
                    E = ep.tile([128, 512], BF16)
                    E_t[b] = E
                    nc.scalar.activation(E[:], psZ_t[b][:], Exp)

                # ---- DVE: P = z*E for block i-3 ----
                if 3 <= i <= NBLK + 2:
                    b3 = i - 3
                    P = pp.tile([128, 512], BF16)
                    P_t[b3] = P
                    nc.vector.scalar_tensor_tensor(P[:], psZ_t[b3][:], 1.0,
                                                   E_t[b3][:], MUL, MUL)

                # ---- GpSimd: chunk load (2 blocks per chunk) ----
                if i < NBLK and i % 2 == 0:
                    c = i // 2
                    xc = xin.tile([128, CR, D], BF16)
                    xc_t[c] = xc
                    nc.gpsimd.dma_start(out=xc[:], in_=xsv[c])

                # ---- PE: 8 transposes for block i ----
                if i < NBLK:
                    c, h = divmod(i, 2)
                    xc = xc_t[c]
                    psT = psTp.tile([128, 8, 128], BF16)
                    for r in range(8):
                        nc.tensor.transpose(psT[:, r, :],
                                            xc[:, 8 * h + r, :], ident[:])
                    xT = xtp.tile([128, 8, 128], BF16)
                    xT_t[i] = xT
                    # ACT evicts 1 tile, DVE 7 (balances engine busy:
                    # ACT copies run 1x, DVE bf16 copies 2x)
                    nc.scalar.copy(xT[:, 0:1, :], psT[:, 0:1, :])
                    nc.vector.tensor_copy(xT[:, 1:8, :], psT[:, 1:8, :])

                # ---- PE: reduce matmuls for block i-4 (S || W col groups) ----
                if 4 <= i <= NBLK + 3:
                    b4 = i - 4
                    g, j = divmod(b4, 4)
                    if j == 0:
                        psSW_t[g] = psSWp.tile([40, 512], F32, name="psSW")
                    psSW = psSW_t[g]
                    # S group occupies array cols 0:8 -> psum rows 0:8,
                    # W group cols 32:40 -> rows 32:40; distinct col groups
                    # run concurrently on the PE. j==0 matmuls write each
                    # region fully (zeros outside their 2 rows), so later
                    # start=False matmuls accumulate onto clean zeros.
                    nc.tensor.matmul(psSW[0:8, :], ind[:, j, :], E_t[b4][:],
                                     start=(j == 0), stop=(j == 3),
                                     tile_position=(0, 0),
                                     skip_group_check=True)
                    nc.tensor.matmul(psSW[32:40, :], ind[:, 4 + j, :],
                                     P_t[b4][:],
                                     start=(j == 0), stop=(j == 3),
                                     tile_position=(0, 32),
                                     skip_group_check=True)
                    if j == 3:
                        nc.scalar.copy(stats_sw[:, g, :], psSW[:])
                        # fan this group's stats onto 128 partitions now,
                        # overlapped with the remaining loop
                        nc.sync.dma_start(out=s128[:, g, :],
                                          in_=stats_sw[0:8, g, :])
                        nc.sync.dma_start(out=w128[:, g, :],
                                          in_=stats_sw[32:40, g, :])

                # free refs we no longer need (python-side bookkeeping only)
                if i >= 5:
                    b5 = i - 5
                    psZ_t.pop(b5, None)
                    E_t.pop(b5, None)
                    xT_t.pop(b5, None)
                    P_t.pop(b5, None)

            s128f = s128[:].rearrange("p g c -> p (g c)")
            w128f = w128[:].rearrange("p g c -> p (g c)")
            lnS = fin.tile([128, 256], F32)
            lsum = fin.tile([128, 1], F32)
            nc.scalar.activation(lnS[:], s128f, Ln, accum_out=lsum[:])
            rS = fin.tile([128, 256], F32)
            nc.scalar.activation(rS[:], lnS[:], Exp, scale=-1.0)
            scr = fin.tile([128, 256], F32)
            wsum = fin.tile([128, 1], F32)
            nc.vector.scalar_tensor_tensor(scr[:], w128f, 1.0, rS[:],
                                           MUL, MUL, accum_out=wsum[:])
            ob = fin.tile([128, 2], F32)
            nc.vector.tensor_copy(ob[:, 0:1], lsum[:])
            nc.vector.tensor_copy(ob[:, 1:2], wsum[:])
            nc.sync.dma_start(out=out_ap[:], in_=ob[:])

    _CACHE["nc"] = nc
    return nc


def _entropy_np(p):
    p = np.where(p <= 0, EPS, p)
    p = np.where(p >= 1, 1.0 - EPS, p)
    return -np.sum(p * np.log(p), axis=-1)


def kernel(x, m):
    nc = _build()

    m2t = (2.0 * np.float64(m).T).astype(ml_dtypes.bfloat16)   # [128, 64]
    ident = np.eye(128, dtype=ml_dtypes.bfloat16)
    ind = np.zeros((128, 8, 8), dtype=ml_dtypes.bfloat16)
    for j in range(4):
        ind[0:64, j, 2 * j] = 1          # S, chunk A (psum rows 0:8)
        ind[64:128, j, 2 * j + 1] = 1    # S, chunk B
        ind[0:64, 4 + j, 2 * j] = 1      # W, chunk A (psum rows 32:40)
        ind[64:128, 4 + j, 2 * j + 1] = 1

    in_maps = []
    for c in range(NCORES):
        in_maps.append({
            "xs": np.ascontiguousarray(x[c * NSHARD:(c + 1) * NSHARD]),
            "m2t": m2t, "ind": ind, "ident": ident,
        })
    _CACHE["last_in_maps"] = in_maps
    res = run_bass_kernel_spmd(nc, in_maps, core_ids=list(range(NCORES)))

    tot_ls = 0.0
    tot_ws = 0.0
    for c in range(NCORES):
        o = np.float64(res.results[c]["out"])
        tot_ls += o[:, 0].sum()
        tot_ws += o[:, 1].sum()
    intra = (tot_ls - tot_ws) / N

    # inter term on host (tiny), replicating the reference exactly
    m64 = np.float64(m)
    mu = m64.mean(axis=0)
    d2 = ((mu[None, :] - m64) ** 2).sum(axis=1)
    zl = -d2
    zl -= zl.max()
    e = np.exp(zl)
    p = e / e.sum()
    inter = _entropy_np(p)

    total = intra - LAMB * inter
    return (np.float32(total), np.float32(intra), np.float32(inter))


# revision 15
# speedup vs baseline: 1.5863x; 1.0071x over previous
"""Trainium2 Bass kernel for the unsupervised-entropy loss.

intra = mean_r H_r where H_r = entropy(softmax(-d2(x_r, m))).
Softmax is shift-invariant, so with unit-norm m rows the logits reduce to
z = 2 x m^T (the ||x||^2 and ||m||^2 terms drop).  Per row:
  S = sum_j exp(z_j),  W = sum_j z_j exp(z_j),  H = log S - W/S

Row-to-partition assignment is contiguous (partition p owns rows
[p*256, (p+1)*256) of the shard) so every DMA chunk is one contiguous
8 KiB read per partition (128 descriptors per chunk, minimal SWDGE cost).
Row order is irrelevant: only sums over all rows are needed.

Software-pipelined device loop, per iteration i (block b = 1024 rows):
  PE : z-matmuls(b=i-1)  [col-tiled pair: chunk A -> psum partitions 0:64,
       chunk B -> 64:128, one accumulation group, concurrent col-groups]
       8 PE transposes(b=i)   x_nat [128p,128d] -> psum xT [128d,128p]
       2 reduce matmuls(b=i-2): indicator lhsT [128,16] accumulating
       S and W into one psum bank [16,512] shared by 4 blocks
  ACT: exp(psZ(i-1)) -> E bf16; evict xT tiles 0:3 psum->sbuf
  DVE: P = z*E (b=i-2); evict xT tiles 3:8; nothing else
  ACT: every 4th block: evict the [16,512] S/W bank -> bf16 staging
  GpSimd: one SWDGE cast-load (f32->bf16) of a 1 MiB chunk every 2 blocks

Final: two SBUF->SBUF DMAs fan the staged [8,8,512] S and W values onto
[128,256]; ACT Ln(+accum lsum), ACT exp(-lnS)=1/S, DVE W*rS(+accum wsum).
Output [128,2] per-partition partial sums; host reduces and adds the
(tiny) inter term.
"""

import json

import numpy as np
import ml_dtypes

import concourse.bass as _bass
import concourse.tile as _tile
from concourse import mybir
from concourse.bass_utils import run_bass_kernel_spmd
from concourse.vector_clock import ScopedClock

F32 = mybir.dt.float32
BF16 = mybir.dt.bfloat16
N, D, K = 262144, 128, 64
NCORES = 8
NSHARD = N // NCORES          # 32768 rows per core
BLK = 1024                    # rows per block
NBLK = NSHARD // BLK          # 32 blocks
NCHUNK = 16                   # DMA chunks (2 blocks each)
RPP = NSHARD // 128           # rows per partition (256)
EPS = 1e-16
LAMB = 1.0


# ---- workarounds: this walrus build rejects >1 sync wait per instruction ----

def _split_multiwait(json_bytes: bytes) -> bytes:
    data = json.loads(json_bytes)
    counter = [0]
    for fn in data["functions"]:
        for blk in fn["blocks"]:
            new_insts = []
            for inst in blk["instructions"]:
                si = inst.get("sync_info")
                waits = (si or {}).get("on_wait") or []
                if len(waits) > 1:
                    for w in waits[:-1]:
                        counter[0] += 1
                        new_insts.append({
                            "debug": inst.get("debug"),
                            "engine": inst["engine"],
                            "ins": [],
                            "name": f"splitw_{counter[0]}_{inst['name']}",
                            "opcode": "EventSemaphore",
                            "outs": [],
                            "sync_info": {"on_update": [], "on_wait": [w]},
                        })
                    si["on_wait"] = [waits[-1]]
                new_insts.append(inst)
            blk["instructions"] = new_insts
    return json.dumps(data).encode()


class PatchedBass(_bass.Bass):
    def to_json_bytes(self) -> bytes:
        return _split_multiwait(super().to_json_bytes())


class SplitDrainTileContext(_tile.TileContext):
    def _drain_and_barrier(self, tick_clock, wait_clock):
        drain_inst = self.nc.sync.drain()
        wait_clock.add_sem_waits(
            drain_inst.ins, ScopedClock({None: tick_clock.global_clock})
        )
        si = drain_inst.ins.sync_info
        if si is not None and len(si.on_wait) > 1:
            waits = list(si.on_wait)
            si.on_wait = waits[:1]
            drain_inst.ins.sync_info = si
            for w in waits[1:]:
                d2 = self.nc.sync.drain()
                si2 = d2.ins.sync_info
                if si2 is None:
                    import copy
                    si2 = copy.copy(si)
                si2.on_wait = [w]
                si2.on_update = []
                d2.ins.sync_info = si2
        self.nc.all_engine_barrier()
        assert self.sems is not None
        popped = self.nc._tile_sem_poison_stack.pop()
        assert popped is self._sem_poison
        self.nc.clear_and_free_semaphores(list(self.sems.allocated().values()))
        self.nc.all_engine_barrier()


# ------------------------------ kernel build ------------------------------

_CACHE = {}


def _build():
    if "nc" in _CACHE:
        return _CACHE["nc"]
    nc = PatchedBass("TRN2", target_bir_lowering=False, debug=False)
    xs_ap = nc.dram_tensor("xs", [NSHARD, D], F32, kind="ExternalInput").ap()
    m2t_ap = nc.dram_tensor("m2t", [D, K], BF16, kind="ExternalInput").ap()
    ind_ap = nc.dram_tensor("ind", [128, 8, 8], BF16, kind="ExternalInput").ap()
    id_ap = nc.dram_tensor("ident", [128, 128], BF16, kind="ExternalInput").ap()
    out_ap = nc.dram_tensor("out", [128, 16], F32, kind="ExternalOutput").ap()

    Exp = mybir.ActivationFunctionType.Exp
    Ln = mybir.ActivationFunctionType.Ln
    MUL = mybir.AluOpType.mult

    # row = p*(NBLK*8) + b*8 + r: partition p owns a contiguous row range,
    # so each per-block load is one contiguous 4 KiB read per partition.
    xsv = xs_ap.rearrange("(p b r) d -> b p r d", p=128, b=NBLK)

    with SplitDrainTileContext(nc) as tc:
        with tc.tile_pool(name="const", bufs=1) as const, \
             tc.tile_pool(name="xin", bufs=8) as xin, \
             tc.tile_pool(name="xtp", bufs=4) as xtp, \
             tc.tile_pool(name="ep", bufs=5) as ep, \
             tc.tile_pool(name="pp", bufs=4) as pp, \
             tc.tile_pool(name="gf", bufs=2) as gf, \
             tc.tile_pool(name="stage", bufs=1) as stage, \
             tc.tile_pool(name="fin", bufs=1) as fin, \
             tc.tile_pool(name="psT", bufs=2, space="PSUM") as psTp, \
             tc.tile_pool(name="psZ", bufs=3, space="PSUM") as psZp, \
             tc.tile_pool(name="psSW", bufs=2, space="PSUM") as psSWp:

            m2t = const.tile([D, K], BF16)
            nc.sync.dma_start(out=m2t[:], in_=m2t_ap[:])
            ind = const.tile([128, 8, 8], BF16)
            nc.sync.dma_start(out=ind[:], in_=ind_ap[:])
            ident = const.tile([128, 128], BF16)
            nc.sync.dma_start(out=ident[:], in_=id_ap[:])

            # staged S/W per 4-block group (rows 0:8 = S, 32:40 = W,
            # rows 8:32 are dead padding so one evict instruction covers
            # both at the same per-partition free-dim cost)
            stats_sw = stage.tile([40, 8, 512], BF16)
            s128 = fin.tile([128, 8, 32], BF16)
            w128 = fin.tile([128, 8, 32], BF16)
            # per-group partial sums: cols 0:8 = sum ln S, 8:16 = sum W/S
            lsw = fin.tile([128, 16], F32)

            xc_t = {}     # block -> xin tile
            xT_t = {}     # block -> xtp tile (sbuf xT)
            psZ_t = {}    # block -> psum z tile
            E_t = {}      # block -> E tile
            P_t = {}      # block -> P tile
            psSW_t = {}   # group -> psum stats tile

            def group_final(g):
                # s128/w128 group g landed >=4 iterations ago via the fan
                # DMAs; fold it into the running per-partition sums.
                lnSg = gf.tile([128, 32], F32, name="lnSg")
                nc.scalar.activation(lnSg[:], s128[:, g, :], Ln,
                                     accum_out=lsw[:, g:g + 1])
                rSg = gf.tile([128, 32], F32, name="rSg")
                nc.scalar.activation(rSg[:], lnSg[:], Exp, scale=-1.0)
                scrg = gf.tile([128, 32], F32, name="scrg")
                nc.vector.scalar_tensor_tensor(
                    scrg[:], w128[:, g, :], 1.0, rSg[:], MUL, MUL,
                    accum_out=lsw[:, 8 + g:9 + g])

            for i in range(NBLK + 6):
                # ---- GpSimd: load block i (prefetch governed by pool) ----
                if i < NBLK:
                    xc = xin.tile([128, 8, D], BF16)
                    xc_t[i] = xc
                    nc.gpsimd.dma_start(out=xc[:], in_=xsv[i])

                # ---- PE: z-matmuls for block i-2 (xT evicted 2 iters ago) ----
                if 2 <= i <= NBLK + 1:
                    b = i - 2
                    xTf = xT_t[b][:].rearrange("d r p -> d (r p)")
                    psZ = psZp.tile([128, 512], F32)
                    psZ_t[b] = psZ
                    nc.tensor.matmul(psZ[0:64, :], m2t[:], xTf[:, 0:512],
                                     start=True, stop=True,
                                     tile_position=(0, 0))
                    nc.tensor.matmul(psZ[64:128, :], m2t[:], xTf[:, 512:1024],
                                     start=True, stop=True,
                                     tile_position=(0, 64))

                # ---- ACT: exp for block i-2 ----
                if 2 <= i <= NBLK + 1:
                    b = i - 2
                    E = ep.tile([128, 512], BF16)
                    E_t[b] = E
                    nc.scalar.activation(E[:], psZ_t[b][:], Exp)

                # ---- DVE: P = z*E for block i-3 ----
                if 3 <= i <= NBLK + 2:
                    b3 = i - 3
                    P = pp.tile([128, 512], BF16)
                    P_t[b3] = P
                    nc.vector.scalar_tensor_tensor(P[:], psZ_t[b3][:], 1.0,
                                                   E_t[b3][:], MUL, MUL)

                # ---- PE: 8 transposes for block i ----
                if i < NBLK:
                    xc = xc_t[i]
                    psT = psTp.tile([128, 8, 128], BF16)
                    for r in range(8):
                        nc.tensor.transpose(psT[:, r, :], xc[:, r, :],
                                            ident[:])
                    xT = xtp.tile([128, 8, 128], BF16)
                    xT_t[i] = xT
                    # ACT evicts 1 tile, DVE 7 (balances engine busy:
                    # ACT copies run 1x, DVE bf16 copies 2x)
                    nc.scalar.copy(xT[:, 0:1, :], psT[:, 0:1, :])
                    nc.vector.tensor_copy(xT[:, 1:8, :], psT[:, 1:8, :])

                # ---- PE: reduce matmuls for block i-5 (S || W col groups) ----
                if 5 <= i <= NBLK + 4:
                    b5 = i - 5
                    g, j = divmod(b5, 4)
                    if j == 0:
                        psSW_t[g] = psSWp.tile([40, 512], F32, name="psSW")
                    psSW = psSW_t[g]
                    # S group occupies array cols 0:8 -> psum rows 0:8,
                    # W group cols 32:40 -> rows 32:40; distinct col groups
                    # run concurrently on the PE. j==0 matmuls write each
                    # region fully (zeros outside their 2 rows), so later
                    # start=False matmuls accumulate onto clean zeros.
                    nc.tensor.matmul(psSW[0:8, :], ind[:, j, :], E_t[b5][:],
                                     start=(j == 0), stop=(j == 3),
                                     tile_position=(0, 0),
                                     skip_group_check=True)
                    nc.tensor.matmul(psSW[32:40, :], ind[:, 4 + j, :],
                                     P_t[b5][:],
                                     start=(j == 0), stop=(j == 3),
                                     tile_position=(0, 32),
                                     skip_group_check=True)
                    if j == 3:
                        nc.scalar.copy(stats_sw[:, g, :], psSW[:])
                        # fan this group's stats onto 128 partitions now,
                        # overlapped with the remaining loop
                        nc.sync.dma_start(out=s128[:, g, :],
                                          in_=stats_sw[0:8, g, :])
                        nc.sync.dma_start(out=w128[:, g, :],
                                          in_=stats_sw[32:40, g, :])
                        if g >= 1:
                            group_final(g - 1)

                # free refs we no longer need (python-side bookkeeping only)
                if i >= 6:
                    b6 = i - 6
                    psZ_t.pop(b6, None)
                    E_t.pop(b6, None)
                    xT_t.pop(b6, None)
                    P_t.pop(b6, None)

            group_final(7)
            nc.sync.dma_start(out=out_ap[:], in_=lsw[:])

    _CACHE["nc"] = nc
    return nc


def _entropy_np(p):
    p = np.where(p <= 0, EPS, p)
    p = np.where(p >= 1, 1.0 - EPS, p)
    return -np.sum(p * np.log(p), axis=-1)


def kernel(x, m):
    nc = _build()

    m2t = (2.0 * np.float64(m).T).astype(ml_dtypes.bfloat16)   # [128, 64]
    ident = np.eye(128, dtype=ml_dtypes.bfloat16)
    ind = np.zeros((128, 8, 8), dtype=ml_dtypes.bfloat16)
    for j in range(4):
        ind[0:64, j, 2 * j] = 1          # S, chunk A (psum rows 0:8)
        ind[64:128, j, 2 * j + 1] = 1    # S, chunk B
        ind[0:64, 4 + j, 2 * j] = 1      # W, chunk A (psum rows 32:40)
        ind[64:128, 4 + j, 2 * j + 1] = 1

    in_maps = []
    for c in range(NCORES):
        in_maps.append({
            "xs": np.ascontiguousarray(x[c * NSHARD:(c + 1) * NSHARD]),
            "m2t": m2t, "ind": ind, "ident": ident,
        })
    _CACHE["last_in_maps"] = in_maps
    res = run_bass_kernel_spmd(nc, in_maps, core_ids=list(range(NCORES)))

    tot_ls = 0.0
    tot_ws = 0.0
    for c in range(NCORES):
        o = np.float64(res.results[c]["out"])
        tot_ls += o[:, 0:8].sum()
        tot_ws += o[:, 8:16].sum()
    intra = (tot_ls - tot_ws) / N

    # inter term on host (tiny), replicating the reference exactly
    m64 = np.float64(m)
    mu = m64.mean(axis=0)
    d2 = ((mu[None, :] - m64) ** 2).sum(axis=1)
    zl = -d2
    zl -= zl.max()
    e = np.exp(zl)
    p = e / e.sum()
    inter = _entropy_np(p)

    total = intra - LAMB * inter
    return (np.float32(total), np.float32(intra), np.float32(inter))


# revision 18
# speedup vs baseline: 1.6533x; 1.0422x over previous
"""Trainium2 Bass kernel for the unsupervised-entropy loss.

intra = mean_r H_r where H_r = entropy(softmax(-d2(x_r, m))).
Softmax is shift-invariant, so with unit-norm m rows the logits reduce to
z = 2 x m^T (the ||x||^2 and ||m||^2 terms drop).  Per row:
  S = sum_j exp(z_j),  W = sum_j z_j exp(z_j),  H = log S - W/S

Row-to-partition assignment is contiguous (partition p owns rows
[p*256, (p+1)*256) of the shard) so every DMA chunk is one contiguous
8 KiB read per partition (128 descriptors per chunk, minimal SWDGE cost).
Row order is irrelevant: only sums over all rows are needed.

Software-pipelined device loop, per iteration i (block b = 1024 rows):
  PE : z-matmuls(b=i-1)  [col-tiled pair: chunk A -> psum partitions 0:64,
       chunk B -> 64:128, one accumulation group, concurrent col-groups]
       8 PE transposes(b=i)   x_nat [128p,128d] -> psum xT [128d,128p]
       2 reduce matmuls(b=i-2): indicator lhsT [128,16] accumulating
       S and W into one psum bank [16,512] shared by 4 blocks
  ACT: exp(psZ(i-1)) -> E bf16; evict xT tiles 0:3 psum->sbuf
  DVE: P = z*E (b=i-2); evict xT tiles 3:8; nothing else
  ACT: every 4th block: evict the [16,512] S/W bank -> bf16 staging
  GpSimd: one SWDGE cast-load (f32->bf16) of a 1 MiB chunk every 2 blocks

Final: two SBUF->SBUF DMAs fan the staged [8,8,512] S and W values onto
[128,256]; ACT Ln(+accum lsum), ACT exp(-lnS)=1/S, DVE W*rS(+accum wsum).
Output [128,2] per-partition partial sums; host reduces and adds the
(tiny) inter term.
"""

import json

import numpy as np
import ml_dtypes

import concourse.bass as _bass
import concourse.tile as _tile
from concourse import mybir
from concourse.bass_utils import run_bass_kernel_spmd
from concourse.vector_clock import ScopedClock

F32 = mybir.dt.float32
BF16 = mybir.dt.bfloat16
N, D, K = 262144, 128, 64
NCORES = 8
NSHARD = N // NCORES          # 32768 rows per core
BLK = 1024                    # rows per block
NBLK = NSHARD // BLK          # 32 blocks
NCHUNK = 16                   # DMA chunks (2 blocks each)
RPP = NSHARD // 128           # rows per partition (256)
EPS = 1e-16
LAMB = 1.0


# ---- workarounds: this walrus build rejects >1 sync wait per instruction ----

def _split_multiwait(json_bytes: bytes) -> bytes:
    data = json.loads(json_bytes)
    counter = [0]
    for fn in data["functions"]:
        for blk in fn["blocks"]:
            new_insts = []
            for inst in blk["instructions"]:
                si = inst.get("sync_info")
                waits = (si or {}).get("on_wait") or []
                if len(waits) > 1:
                    for w in waits[:-1]:
                        counter[0] += 1
                        new_insts.append({
                            "debug": inst.get("debug"),
                            "engine": inst["engine"],
                            "ins": [],
                            "name": f"splitw_{counter[0]}_{inst['name']}",
                            "opcode": "EventSemaphore",
                            "outs": [],
                            "sync_info": {"on_update": [], "on_wait": [w]},
                        })
                    si["on_wait"] = [waits[-1]]
                new_insts.append(inst)
            blk["instructions"] = new_insts
    return json.dumps(data).encode()


class PatchedBass(_bass.Bass):
    def to_json_bytes(self) -> bytes:
        return _split_multiwait(super().to_json_bytes())


class SplitDrainTileContext(_tile.TileContext):
    def _drain_and_barrier(self, tick_clock, wait_clock):
        drain_inst = self.nc.sync.drain()
        wait_clock.add_sem_waits(
            drain_inst.ins, ScopedClock({None: tick_clock.global_clock})
        )
        si = drain_inst.ins.sync_info
        if si is not None and len(si.on_wait) > 1:
            waits = list(si.on_wait)
            si.on_wait = waits[:1]
            drain_inst.ins.sync_info = si
            for w in waits[1:]:
                d2 = self.nc.sync.drain()
                si2 = d2.ins.sync_info
                if si2 is None:
                    import copy
                    si2 = copy.copy(si)
                si2.on_wait = [w]
                si2.on_update = []
                d2.ins.sync_info = si2
        self.nc.all_engine_barrier()
        assert self.sems is not None
        popped = self.nc._tile_sem_poison_stack.pop()
        assert popped is self._sem_poison
        self.nc.clear_and_free_semaphores(list(self.sems.allocated().values()))
        self.nc.all_engine_barrier()


# ------------------------------ kernel build ------------------------------

_CACHE = {}


def _build():
    if "nc" in _CACHE:
        return _CACHE["nc"]
    nc = PatchedBass("TRN2", target_bir_lowering=False, debug=False)
    xs_ap = nc.dram_tensor("xs", [NSHARD, D], F32, kind="ExternalInput").ap()
    m2t_ap = nc.dram_tensor("m2t", [D, K], BF16, kind="ExternalInput").ap()
    ind_ap = nc.dram_tensor("ind", [128, 8, 8], BF16, kind="ExternalInput").ap()
    id_ap = nc.dram_tensor("ident", [128, 128], BF16, kind="ExternalInput").ap()
    out_ap = nc.dram_tensor("out", [128, 16], F32, kind="ExternalOutput").ap()

    Exp = mybir.ActivationFunctionType.Exp
    Ln = mybir.ActivationFunctionType.Ln
    MUL = mybir.AluOpType.mult

    # row = p*(NBLK*8) + b*8 + r: partition p owns a contiguous row range,
    # so each per-block load is one contiguous 4 KiB read per partition.
    xsv = xs_ap.rearrange("(p b r) d -> b p r d", p=128, b=NBLK)

    with SplitDrainTileContext(nc) as tc:
        with tc.tile_pool(name="const", bufs=1) as const, \
             tc.tile_pool(name="xin", bufs=8) as xin, \
             tc.tile_pool(name="xtp", bufs=4) as xtp, \
             tc.tile_pool(name="ep", bufs=5) as ep, \
             tc.tile_pool(name="pp", bufs=4) as pp, \
             tc.tile_pool(name="gf", bufs=2) as gf, \
             tc.tile_pool(name="stage", bufs=1) as stage, \
             tc.tile_pool(name="fin", bufs=1) as fin, \
             tc.tile_pool(name="psT", bufs=2, space="PSUM") as psTp, \
             tc.tile_pool(name="psZ", bufs=3, space="PSUM") as psZp, \
             tc.tile_pool(name="psSW", bufs=2, space="PSUM") as psSWp:

            m2t = const.tile([D, K], BF16)
            nc.sync.dma_start(out=m2t[:], in_=m2t_ap[:])
            ind = const.tile([128, 8, 8], BF16)
            nc.sync.dma_start(out=ind[:], in_=ind_ap[:])
            ident = const.tile([128, 128], BF16)
            nc.sync.dma_start(out=ident[:], in_=id_ap[:])

            # staged S/W per 4-block group (rows 0:8 = S, 32:40 = W,
            # rows 8:32 are dead padding so one evict instruction covers
            # both at the same per-partition free-dim cost)
            stats_sw = stage.tile([40, 8, 512], BF16)
            s128 = fin.tile([128, 8, 32], BF16)
            w128 = fin.tile([128, 8, 32], BF16)
            # per-group partial sums: cols 0:8 = sum ln S, 8:16 = sum W/S
            lsw = fin.tile([128, 16], F32)

            xc_t = {}     # block -> xin tile
            xT_t = {}     # block -> xtp tile (sbuf xT)
            psZ_t = {}    # block -> psum z tile
            E_t = {}      # block -> E tile
            P_t = {}      # block -> P tile
            psSW_t = {}   # group -> psum stats tile

            def group_final(g):
                # s128/w128 group g landed >=4 iterations ago via the fan
                # DMAs; fold it into the running per-partition sums.
                lnSg = gf.tile([128, 32], F32, name="lnSg")
                nc.scalar.activation(lnSg[:], s128[:, g, :], Ln,
                                     accum_out=lsw[:, g:g + 1])
                rSg = gf.tile([128, 32], F32, name="rSg")
                nc.scalar.activation(rSg[:], lnSg[:], Exp, scale=-1.0)
                scrg = gf.tile([128, 32], F32, name="scrg")
                nc.vector.scalar_tensor_tensor(
                    scrg[:], w128[:, g, :], 1.0, rSg[:], MUL, MUL,
                    accum_out=lsw[:, 8 + g:9 + g])

            for i in range(NBLK + 6):
                # ---- GpSimd: load block i (prefetch governed by pool) ----
                if i < NBLK:
                    xc = xin.tile([128, 8, D], BF16)
                    xc_t[i] = xc
                    nc.gpsimd.dma_start(out=xc[:], in_=xsv[i])

                # ---- PE: z-matmuls for block i-2 (xT evicted 2 iters ago) ----
                if 2 <= i <= NBLK + 1:
                    b = i - 2
                    xTf = xT_t[b][:].rearrange("d r p -> d (r p)")
                    psZ = psZp.tile([128, 512], F32)
                    psZ_t[b] = psZ
                    nc.tensor.matmul(psZ[0:64, :], m2t[:], xTf[:, 0:512],
                                     start=True, stop=True,
                                     tile_position=(0, 0))
                    nc.tensor.matmul(psZ[64:128, :], m2t[:], xTf[:, 512:1024],
                                     start=True, stop=True,
                                     tile_position=(0, 64))

                # ---- ACT: exp for block i-2 ----
                if 2 <= i <= NBLK + 1:
                    b = i - 2
                    E = ep.tile([128, 512], BF16)
                    E_t[b] = E
                    nc.scalar.activation(E[:], psZ_t[b][:], Exp)

                # ---- DVE: P = z*E for block i-3 ----
                if 3 <= i <= NBLK + 2:
                    b3 = i - 3
                    P = pp.tile([128, 512], BF16)
                    P_t[b3] = P
                    nc.vector.scalar_tensor_tensor(P[:], psZ_t[b3][:], 1.0,
                                                   E_t[b3][:], MUL, MUL)

                # ---- PE: 8 transposes for block i ----
                if i < NBLK:
                    xc = xc_t[i]
                    psT = psTp.tile([128, 8, 128], BF16)
                    for r in range(8):
                        nc.tensor.transpose(psT[:, r, :], xc[:, r, :],
                                            ident[:])
                    xT = xtp.tile([128, 8, 128], BF16)
                    xT_t[i] = xT
                    # all 8 tiles on DVE: its bf16 2x copy beats splitting
                    # (ACT's ~300ns per-instruction fixed cost dominates)
                    nc.vector.tensor_copy(xT[:], psT[:])

                # ---- PE: reduce matmuls for block i-5 (S || W col groups) ----
                if 5 <= i <= NBLK + 4:
                    b5 = i - 5
                    g, j = divmod(b5, 4)
                    if j == 0:
                        psSW_t[g] = psSWp.tile([40, 512], F32, name="psSW")
                    psSW = psSW_t[g]
                    # S group occupies array cols 0:8 -> psum rows 0:8,
                    # W group cols 32:40 -> rows 32:40; distinct col groups
                    # run concurrently on the PE. j==0 matmuls write each
                    # region fully (zeros outside their 2 rows), so later
                    # start=False matmuls accumulate onto clean zeros.
                    nc.tensor.matmul(psSW[0:8, :], ind[:, j, :], E_t[b5][:],
                                     start=(j == 0), stop=(j == 3),
                                     tile_position=(0, 0),
                                     skip_group_check=True)
                    nc.tensor.matmul(psSW[32:40, :], ind[:, 4 + j, :],
                                     P_t[b5][:],
                                     start=(j == 0), stop=(j == 3),
                                     tile_position=(0, 32),
                                     skip_group_check=True)
                    if j == 3:
                        nc.scalar.copy(stats_sw[:, g, :], psSW[:])
                        # fan this group's stats onto 128 partitions now,
                        # overlapped with the remaining loop
                        nc.sync.dma_start(out=s128[:, g, :],
                                          in_=stats_sw[0:8, g, :])
                        nc.sync.dma_start(out=w128[:, g, :],
                                          in_=stats_sw[32:40, g, :])
                        # finals lag 2 groups so the fan DMA has ~15us to
                        # land; a 1-group lag head-of-line blocks ACT on
                        # the DMA completion latency
                        if g >= 2:
                            group_final(g - 2)

                # free refs we no longer need (python-side bookkeeping only)
                if i >= 6:
                    b6 = i - 6
                    psZ_t.pop(b6, None)
                    E_t.pop(b6, None)
                    xT_t.pop(b6, None)
                    P_t.pop(b6, None)

            group_final(6)
            group_final(7)
            nc.sync.dma_start(out=out_ap[:], in_=lsw[:])

    _CACHE["nc"] = nc
    return nc


def _entropy_np(p):
    p = np.where(p <= 0, EPS, p)
    p = np.where(p >= 1, 1.0 - EPS, p)
    return -np.sum(p * np.log(p), axis=-1)


def kernel(x, m):
    nc = _build()

    m2t = (2.0 * np.float64(m).T).astype(ml_dtypes.bfloat16)   # [128, 64]
    ident = np.eye(128, dtype=ml_dtypes.bfloat16)
    ind = np.zeros((128, 8, 8), dtype=ml_dtypes.bfloat16)
    for j in range(4):
        ind[0:64, j, 2 * j] = 1          # S, chunk A (psum rows 0:8)
        ind[64:128, j, 2 * j + 1] = 1    # S, chunk B
        ind[0:64, 4 + j, 2 * j] = 1      # W, chunk A (psum rows 32:40)
        ind[64:128, 4 + j, 2 * j + 1] = 1

    in_maps = []
    for c in range(NCORES):
        in_maps.append({
            "xs": np.ascontiguousarray(x[c * NSHARD:(c + 1) * NSHARD]),
            "m2t": m2t, "ind": ind, "ident": ident,
        })
    _CACHE["last_in_maps"] = in_maps
    res = run_bass_kernel_spmd(nc, in_maps, core_ids=list(range(NCORES)))

    tot_ls = 0.0
    tot_ws = 0.0
    for c in range(NCORES):
        o = np.float64(res.results[c]["out"])
        tot_ls += o[:, 0:8].sum()
        tot_ws += o[:, 8:16].sum()
    intra = (tot_ls - tot_ws) / N

    # inter term on host (tiny), replicating the reference exactly
    m64 = np.float64(m)
    mu = m64.mean(axis=0)
    d2 = ((mu[None, :] - m64) ** 2).sum(axis=1)
    zl = -d2
    zl -= zl.max()
    e = np.exp(zl)
    p = e / e.sum()
    inter = _entropy_np(p)

    total = intra - LAMB * inter
    return (np.float32(total), np.float32(intra), np.float32(inter))
